# revision 1
# baseline (speedup 1.0000x reference)
"""Trainium2 Bass kernel for nn_CombineConcat (pairwise broadcast+concat).

reference semantics (per batch b):
  out[b, i*N + j, 0:D]   = x1[b, i, :]
  out[b, i*N + j, D:2*D] = x2[b, j, :]

Shapes (hardcoded): x1, x2 = [16, 128, 256] f32 -> out = [16, 16384, 512] f32.

Strategy: data-parallel over the batch dim, 2 batches per core on 8 cores.
The kernel is pure data movement and write-bandwidth bound (each core writes
64 MB, reads 256 KB).  Inputs are loaded to SBUF once; the full output is
generated with broadcast (stride-0) DMA reads from SBUF, written straight to
HBM — no compute engines involved.
"""

import numpy as np

_B, _N, _D = 16, 128, 256
_NCORES = 8
_BPC = _B // _NCORES  # batches per core

_NC_CACHE = {}


def _build_nc():
    import concourse.bacc as bacc
    import concourse.mybir as mybir
    from concourse.tile import TileContext

    nc = bacc.Bacc("TRN2", target_bir_lowering=False)
    x1 = nc.dram_tensor("x1", [_BPC, _N, _D], mybir.dt.float32, kind="ExternalInput")
    x2 = nc.dram_tensor("x2", [_BPC, _N, _D], mybir.dt.float32, kind="ExternalInput")
    out = nc.dram_tensor(
        "out", [_BPC, _N * _N, 2 * _D], mybir.dt.float32, kind="ExternalOutput"
    )

    with TileContext(nc) as tc:
        with tc.tile_pool(name="inp", bufs=_BPC) as pool:
            for b in range(_BPC):
                t1 = pool.tile([_N, _D], mybir.dt.float32)
                t2 = pool.tile([_N, _D], mybir.dt.float32)
                nc.sync.dma_start(out=t1[:], in_=x1[b])
                nc.scalar.dma_start(out=t2[:], in_=x2[b])
                o3 = out[b].rearrange("(i j) d -> i j d", j=_N)  # [N, N, 2D]
                # x1 half: out[b, i*N+j, 0:D] = x1[b, i, :] (broadcast over j).
                # SBUF src: partition i, free-dim AP repeats the row N times
                # (stride 0); DRAM dst: [i, j, d] with d contiguous.
                nc.sync.dma_start(
                    out=o3[:, :, 0:_D],
                    in_=t1[:].unsqueeze(1).broadcast_to([_N, _N, _D]),
                )
                # x2 half: out[b, i*N+j, D:2D] = x2[b, j, :] (broadcast over i).
                # SBUF src: partition j; DRAM dst transposed so partition j's
                # row lands at column-slice [D:2D] of every i-block.
                nc.scalar.dma_start(
                    out=o3.transpose([1, 0, 2])[:, :, _D : 2 * _D],
                    in_=t2[:].unsqueeze(1).broadcast_to([_N, _N, _D]),
                )
    nc.finalize()
    return nc


def _get_nc():
    if "nc" not in _NC_CACHE:
        _NC_CACHE["nc"] = _build_nc()
    return _NC_CACHE["nc"]


def _run(x1, x2, trace=False):
    """Run the kernel on 8 cores; returns (output, BassKernelResults)."""
    from concourse.bass_utils import run_bass_kernel_spmd

    nc = _get_nc()
    x1 = np.ascontiguousarray(np.asarray(x1, dtype=np.float32))
    x2 = np.ascontiguousarray(np.asarray(x2, dtype=np.float32))
    in_maps = [
        {
            "x1": x1[c * _BPC : (c + 1) * _BPC],
            "x2": x2[c * _BPC : (c + 1) * _BPC],
        }
        for c in range(_NCORES)
    ]
    res = run_bass_kernel_spmd(
        nc, in_maps, core_ids=list(range(_NCORES)), trace=trace
    )
    out = np.concatenate([r["out"] for r in res.results], axis=0)
    return out, res


def kernel(x1, x2):
    out, _ = _run(x1, x2, trace=False)
    return out
